# revision 2
# baseline (speedup 1.0000x reference)
"""Trainium2 Bass kernel for nn_EqLinear (S5 permutation-equivariant linear layer).

The layer is y = x @ M with M [1664,1664] built by scattering the small
per-signature weight blocks.  We exploit the equivariant weight-sharing to
factor M through auxiliary sums:

    y = [x_perm, aux] @ Mr,   aux = x_perm @ A  (0/1 block sums:
        T[a] = sum of x2 perm-blocks containing a;  S1 = sum of x1 blocks)

Mr is block-sparse: only 67 of 208 [128,128] tiles are nonzero (vs 169 dense
M tiles), and A adds 21 tiles, so the tensor engine does 88 matmuls per
512-batch chunk instead of 169 — below the HBM-traffic roofline.

Sharding: batch (16384) split across 8 cores; each core runs
yT_shard = Mr.T @ [xT_shard; aux] in float32r (full-rate fp32 PE mode).
Host does only layout work (block permute / transpose / shard) plus
weight-table construction; all O(batch) FLOPs run on device.
"""

import math
import sys
from itertools import permutations

import numpy as np

for _p in ("/opt/trn_rl_repo", "/root/.axon_site/_ro/trn_rl_repo"):
    if _p not in sys.path:
        sys.path.append(_p)

# ---------------------------------------------------------------- constants
N = 5
RADIUS = 2
B = 16384
N_CORES = 8
B_SHARD = B // N_CORES          # 2048
DIM = 1664                      # (1 + 5 + 20) * 64
NBLK = DIM // 64                # 26 perm-blocks of 64 channels
KT_X = 13                       # x k-tiles (26 blocks / 2)
KT_AUX = 3                      # aux k-tiles ([T0;T1],[T2;T3],[T4;S1])
KT = KT_X + KT_AUX              # 16 total contraction tiles
OT = 13                         # output tiles (26 y-blocks / 2)
BC = B_SHARD // 512             # 4 batch chunks of 512


def _overlap(s, t):
    inv = {v: j for j, v in enumerate(t)}
    return tuple(i * 10 + inv[i] for i in s if i in inv)


def _build_kernels():
    kernels = {}
    for len2 in range(3):
        for len1 in range(3):
            if abs(len1 - len2) > RADIUS:
                continue
            wm, widx, nbrs = {}, [], []
            for perm2 in permutations(range(N), len2):
                wid, nb = [], []
                for i1, perm1 in enumerate(permutations(range(N), len1)):
                    sig = _overlap(perm1, perm2)
                    if len1 + len2 - 2 * len(sig) > RADIUS:
                        continue
                    if sig not in wm:
                        wm[sig] = len(wm)
                    wid.append(wm[sig])
                    nb.append(i1)
                widx.append(wid)
                nbrs.append(nb)
            kernels[(len1, len2)] = (len(wm), np.asarray(widx, np.int32),
                                     np.asarray(nbrs, np.int32))
    return kernels


KERNELS = _build_kernels()
PERM1 = list(permutations(range(N), 1))
PERM2 = list(permutations(range(N), 2))
P2I = {p: i for i, p in enumerate(PERM2)}

# ------------------------------------------------------------- block orders
UPAIRS = [(a, b) for a in range(N) for b in range(a + 1, N)]
X_ORDER = [('x1', 0), ('x1', 1), ('x1', 2), ('x1', 3), ('x1', 4), ('x0',)]
for (a, b) in UPAIRS:
    X_ORDER += [('x2', (a, b)), ('x2', (b, a))]
AUX_ORDER = [('T', 0), ('T', 1), ('T', 2), ('T', 3), ('T', 4), ('S1',)]
ROW_ORDER = X_ORDER + AUX_ORDER
Y_ORDER = [('y0',), ('y1', 0), ('y1', 1), ('y1', 2), ('y1', 3), ('y1', 4)]
for (a, b) in UPAIRS:
    Y_ORDER += [('y2', (a, b)), ('y2', (b, a))]

XI = {b: i for i, b in enumerate(X_ORDER)}
RI = {b: i for i, b in enumerate(ROW_ORDER)}
YI = {b: i for i, b in enumerate(Y_ORDER)}


def _xblk_ref(nm):
    if nm[0] == 'x0':
        return 0
    if nm[0] == 'x1':
        return 1 + nm[1]
    return 6 + P2I[nm[1]]


def _yblk_ref(nm):
    if nm[0] == 'y0':
        return 0
    if nm[0] == 'y1':
        return 1 + nm[1]
    return 6 + P2I[nm[1]]


# column permutations (in units of 64-blocks)
X_PERM_COLS = np.concatenate(
    [np.arange(64) + 64 * _xblk_ref(nm) for nm in X_ORDER])      # x -> x_perm
Y_INV_COLS = np.empty(DIM, dtype=np.int64)                        # y_perm -> y
for i, nm in enumerate(Y_ORDER):
    Y_INV_COLS[_yblk_ref(nm) * 64: _yblk_ref(nm) * 64 + 64] = \
        np.arange(64) + 64 * i

# aux structure: block-level 0/1 matrix over X_ORDER x AUX_ORDER
A_BLK = np.zeros((NBLK, 6), dtype=np.float32)
for i, nm in enumerate(X_ORDER):
    if nm[0] == 'x2':
        a, b = nm[1]
        A_BLK[i, AUX_ORDER.index(('T', a))] = 1
        A_BLK[i, AUX_ORDER.index(('T', b))] = 1
    elif nm[0] == 'x1':
        A_BLK[i, AUX_ORDER.index(('S1',))] = 1


def _build_Mr(weights):
    """Restructured matrix over ROW_ORDER x Y_ORDER (64-blocks), plus the
    value-independent structure set {(row_blk, col_blk)}."""
    Mr = np.zeros((len(ROW_ORDER) * 64, len(Y_ORDER) * 64), dtype=np.float32)
    structure = set()

    def add(rname, cname, blk):
        r, c = RI[rname] * 64, YI[cname] * 64
        Mr[r:r + 64, c:c + 64] += blk
        structure.add((RI[rname], YI[cname]))

    wT = {k: np.swapaxes(np.asarray(weights[k], np.float32), 1, 2)
          for k in weights}

    add(('x0',), ('y0',), wT[(0, 0)][0])
    add(('S1',), ('y0',), wT[(1, 0)][0])
    for a in range(N):
        add(('T', a), ('y0',), wT[(2, 0)][0] / 2)   # S2 == (1/2) sum_a T_a
        add(('x0',), ('y1', a), wT[(0, 1)][0])
    for p in PERM2:
        add(('x0',), ('y2', p), wT[(0, 2)][0])

    _W, widx, nbrs = KERNELS[(1, 1)]
    for a in range(N):
        row_w = {PERM1[nbrs[a, k]][0]: widx[a, k] for k in range(nbrs.shape[1])}
        offs = {row_w[p] for p in range(N) if p != a}
        assert len(offs) == 1
        woff = wT[(1, 1)][offs.pop()]
        add(('x1', a), ('y1', a), wT[(1, 1)][row_w[a]] - woff)
        add(('S1',), ('y1', a), woff)

    _W, widx, nbrs = KERNELS[(2, 1)]
    for a in range(N):
        wids = set(widx[a])
        assert len(wids) == 1
        assert {PERM2[i] for i in nbrs[a]} == {p for p in PERM2 if a in p}
        add(('T', a), ('y1', a), wT[(2, 1)][wids.pop()])

    _W, widx, nbrs = KERNELS[(1, 2)]
    for i2, (a, b) in enumerate(PERM2):
        for k in range(nbrs.shape[1]):
            c = PERM1[nbrs[i2, k]][0]
            assert c in (a, b)
            add(('x1', c), ('y2', (a, b)), wT[(1, 2)][widx[i2, k]])

    _W, widx, nbrs = KERNELS[(2, 2)]
    for i2, (a, b) in enumerate(PERM2):
        w_a = w_b = w_id = w_sw = None
        for k in range(nbrs.shape[1]):
            p1, wid = PERM2[nbrs[i2, k]], widx[i2, k]
            if p1 == (a, b):
                w_id = wid
            elif p1 == (b, a):
                w_sw = wid
            elif a in p1:
                assert b not in p1 and w_a in (None, wid)
                w_a = wid
            else:
                assert b in p1 and w_b in (None, wid)
                w_b = wid
        Wa, Wb = wT[(2, 2)][w_a], wT[(2, 2)][w_b]
        add(('x2', (a, b)), ('y2', (a, b)), wT[(2, 2)][w_id] - Wa - Wb)
        add(('x2', (b, a)), ('y2', (a, b)), wT[(2, 2)][w_sw] - Wa - Wb)
        add(('T', a), ('y2', (a, b)), Wa)
        add(('T', b), ('y2', (a, b)), Wb)
    return Mr, structure


# static schedule (value-independent): which [128,128] tiles are nonzero
def _static_schedule():
    rng_w = {k: np.random.default_rng(1).standard_normal((W, 64, 64))
             for k, (W, _, _) in KERNELS.items()}
    _, structure = _build_Mr(rng_w)
    # aux tiles: (k in 0..KT_X-1, a in 0..KT_AUX-1) with any A_BLK support
    aux_tiles = []
    for k in range(KT_X):
        for a in range(KT_AUX):
            sub = A_BLK[2 * k:2 * k + 2, 2 * a:2 * a + 2]
            if sub.any():
                aux_tiles.append((k, a))
    # main tiles: (k in 0..KT-1, o) with structure support
    main_k = [[] for _ in range(OT)]
    for (rb, cb) in structure:
        k, o = rb // 2, cb // 2
        if k not in main_k[o]:
            main_k[o].append(k)
    for o in range(OT):
        main_k[o].sort()          # x k-tiles first, aux k-tiles (13..15) last
    return aux_tiles, main_k


AUX_TILES, MAIN_K = _static_schedule()
N_TILES = len(AUX_TILES) + sum(len(v) for v in MAIN_K)


# ---------------------------------------------------------------- bass build
_CACHE = {}


def _build_bass():
    if "nc" in _CACHE:
        return _CACHE["nc"]

    from concourse import bacc, mybir, tile

    f32 = mybir.dt.float32
    f32r = mybir.dt.float32r

    nc = bacc.Bacc("TRN2", target_bir_lowering=False, debug=False,
                   num_devices=N_CORES)
    xt = nc.dram_tensor("xt", [DIM, B_SHARD], f32r, kind="ExternalInput").ap()
    mt = nc.dram_tensor("mt", [128, N_TILES * 128], f32r,
                        kind="ExternalInput").ap()
    yt = nc.dram_tensor("yt", [DIM, B_SHARD], f32, kind="ExternalOutput").ap()

    xt_r = xt.rearrange("(k p) c -> p k c", p=128)   # [128, KT_X, B_SHARD]
    yt_r = yt.rearrange("(o p) c -> p o c", p=128)   # [128, OT, B_SHARD]

    # tile index within mt for each scheduled matmul
    tidx = {}
    ti = 0
    for (k, a) in AUX_TILES:
        tidx[("aux", k, a)] = ti
        ti += 1
    for o in range(OT):
        for k in MAIN_K[o]:
            tidx[("main", k, o)] = ti
            ti += 1

    with tile.TileContext(nc) as tc:
        with (
            tc.tile_pool(name="mpool", bufs=1) as mpool,
            tc.tile_pool(name="xpool", bufs=2 * KT_X) as xpool,
            tc.tile_pool(name="apool", bufs=2 * KT_AUX) as apool,
            tc.tile_pool(name="ypool", bufs=4) as ypool,
            tc.tile_pool(name="psa", bufs=KT_AUX, space="PSUM") as psa_pool,
            tc.tile_pool(name="psm", bufs=4, space="PSUM") as psm_pool,
        ):
            m_sb = mpool.tile([128, N_TILES * 128], f32r)
            nc.sync.dma_start(m_sb[:], mt)

            def lhsT(key):
                t = tidx[key]
                return m_sb[:, t * 128:(t + 1) * 128]

            for bc in range(BC):
                cs = slice(bc * 512, (bc + 1) * 512)
                x_sb = [xpool.tile([128, 512], f32r, tag="x") for _ in range(KT_X)]
                for k in range(KT_X):
                    nc.sync.dma_start(x_sb[k][:], xt_r[:, k, cs])

                # aux sums via 0/1 matmuls
                aux_ps = [psa_pool.tile([128, 512], f32, tag="auxps")
                          for _ in range(KT_AUX)]
                by_a = [[k for (k, a2) in AUX_TILES if a2 == a]
                        for a in range(KT_AUX)]
                for a in range(KT_AUX):
                    ks = by_a[a]
                    for i, k in enumerate(ks):
                        nc.tensor.matmul(
                            aux_ps[a][:], lhsT=lhsT(("aux", k, a)),
                            rhs=x_sb[k][:],
                            start=(i == 0), stop=(i == len(ks) - 1),
                        )
                aux_sb = []
                for a in range(KT_AUX):
                    t = apool.tile([128, 512], f32, tag="aux")
                    nc.vector.tensor_copy(out=t[:], in_=aux_ps[a][:])
                    aux_sb.append(t)

                def rhs(k):
                    if k < KT_X:
                        return x_sb[k][:]
                    return aux_sb[k - KT_X][:].bitcast(f32r)

                for o in range(OT):
                    ks = MAIN_K[o]
                    ps = psm_pool.tile([128, 512], f32, tag="ps")
                    for i, k in enumerate(ks):
                        nc.tensor.matmul(
                            ps[:], lhsT=lhsT(("main", k, o)), rhs=rhs(k),
                            start=(i == 0), stop=(i == len(ks) - 1),
                        )
                    y_sb = ypool.tile([128, 512], f32, tag="y")
                    nc.any.tensor_copy(out=y_sb[:], in_=ps[:])
                    nc.sync.dma_start(yt_r[:, o, cs], y_sb[:])
    nc.compile()
    _CACHE["nc"] = nc
    return nc


# ---------------------------------------------------------------- entry point
def kernel(x, w_0_0, w_1_0, w_2_0, w_0_1, w_1_1, w_2_1, w_0_2, w_1_2, w_2_2,
           _trace=False):
    from concourse import bass_utils

    weights = {(0, 0): w_0_0, (1, 0): w_1_0, (2, 0): w_2_0,
               (0, 1): w_0_1, (1, 1): w_1_1, (2, 1): w_2_1,
               (0, 2): w_0_2, (1, 2): w_1_2, (2, 2): w_2_2}
    Mr, _ = _build_Mr(weights)

    # pack scheduled lhsT tiles: mt[p, t*128+q]
    AE = np.kron(A_BLK, np.eye(64, dtype=np.float32))     # [DIM, 384]
    tiles = []
    for (k, a) in AUX_TILES:
        tiles.append(AE[k * 128:(k + 1) * 128, a * 128:(a + 1) * 128])
    for o in range(OT):
        for k in MAIN_K[o]:
            tiles.append(Mr[k * 128:(k + 1) * 128, o * 128:(o + 1) * 128])
    mt_host = np.ascontiguousarray(np.concatenate(tiles, axis=1))

    x = np.asarray(x, np.float32)
    xT = np.ascontiguousarray(x[:, X_PERM_COLS].T)        # [DIM, B]

    nc = _build_bass()
    in_maps = [
        {"xt": np.ascontiguousarray(xT[:, c * B_SHARD:(c + 1) * B_SHARD]),
         "mt": mt_host}
        for c in range(N_CORES)
    ]
    res = bass_utils.run_bass_kernel_spmd(
        nc, in_maps, core_ids=list(range(N_CORES)), trace=_trace,
    )
    y = np.empty((B, DIM), dtype=np.float32)
    for c in range(N_CORES):
        y[c * B_SHARD:(c + 1) * B_SHARD, :] = \
            res.results[c]["yt"][Y_INV_COLS, :].T
    if _trace:
        kernel.last_results = res
    return y


# revision 6
# speedup vs baseline: 1.6283x; 1.6283x over previous
"""Trainium2 Bass kernel for nn_EqLinear (S5 permutation-equivariant linear layer).

The layer is y = x @ M with M [1664,1664] built by scattering the small
per-signature weight blocks.  We exploit the equivariant weight-sharing to
factor M through auxiliary sums:

    y = [x_perm, aux] @ Mr,   aux = x_perm @ A  (0/1 block sums:
        T[a] = sum of x2 perm-blocks containing a;  S1 = sum of x1 blocks)

Mr is block-sparse: only 67 of 208 [128,128] tiles are nonzero (vs 169 dense
M tiles), and A adds 21 tiles, so the tensor engine does 88 matmuls per
512-batch chunk instead of 169 — below the HBM-traffic roofline.

Sharding: batch (16384) split across 8 cores; each core runs
yT_shard = Mr.T @ [xT_shard; aux] in float32r (full-rate fp32 PE mode).
Host does only layout work (block permute / transpose / shard) plus
weight-table construction; all O(batch) FLOPs run on device.
"""

import math
import sys
from itertools import permutations

import numpy as np

for _p in ("/opt/trn_rl_repo", "/root/.axon_site/_ro/trn_rl_repo"):
    if _p not in sys.path:
        sys.path.append(_p)

# ---------------------------------------------------------------- constants
N = 5
RADIUS = 2
B = 16384
N_CORES = 8
B_SHARD = B // N_CORES          # 2048
DIM = 1664                      # (1 + 5 + 20) * 64
NBLK = DIM // 64                # 26 perm-blocks of 64 channels
KT_X = 13                       # x k-tiles (26 blocks / 2)
KT_AUX = 3                      # aux k-tiles ([T0;T1],[T2;T3],[T4;S1])
KT = KT_X + KT_AUX              # 16 total contraction tiles
OT = 13                         # output tiles (26 y-blocks / 2)
BC = B_SHARD // 512             # 4 batch chunks of 512


def _overlap(s, t):
    inv = {v: j for j, v in enumerate(t)}
    return tuple(i * 10 + inv[i] for i in s if i in inv)


def _build_kernels():
    kernels = {}
    for len2 in range(3):
        for len1 in range(3):
            if abs(len1 - len2) > RADIUS:
                continue
            wm, widx, nbrs = {}, [], []
            for perm2 in permutations(range(N), len2):
                wid, nb = [], []
                for i1, perm1 in enumerate(permutations(range(N), len1)):
                    sig = _overlap(perm1, perm2)
                    if len1 + len2 - 2 * len(sig) > RADIUS:
                        continue
                    if sig not in wm:
                        wm[sig] = len(wm)
                    wid.append(wm[sig])
                    nb.append(i1)
                widx.append(wid)
                nbrs.append(nb)
            kernels[(len1, len2)] = (len(wm), np.asarray(widx, np.int32),
                                     np.asarray(nbrs, np.int32))
    return kernels


KERNELS = _build_kernels()
PERM1 = list(permutations(range(N), 1))
PERM2 = list(permutations(range(N), 2))
P2I = {p: i for i, p in enumerate(PERM2)}

# ------------------------------------------------------------- block orders
UPAIRS = [(a, b) for a in range(N) for b in range(a + 1, N)]
X_ORDER = [('x1', 0), ('x1', 1), ('x1', 2), ('x1', 3), ('x1', 4), ('x0',)]
for (a, b) in UPAIRS:
    X_ORDER += [('x2', (a, b)), ('x2', (b, a))]
AUX_ORDER = [('T', 0), ('T', 1), ('T', 2), ('T', 3), ('T', 4), ('S1',)]
ROW_ORDER = X_ORDER + AUX_ORDER
Y_ORDER = [('y0',), ('y1', 0), ('y1', 1), ('y1', 2), ('y1', 3), ('y1', 4)]
for (a, b) in UPAIRS:
    Y_ORDER += [('y2', (a, b)), ('y2', (b, a))]

XI = {b: i for i, b in enumerate(X_ORDER)}
RI = {b: i for i, b in enumerate(ROW_ORDER)}
YI = {b: i for i, b in enumerate(Y_ORDER)}


def _xblk_ref(nm):
    if nm[0] == 'x0':
        return 0
    if nm[0] == 'x1':
        return 1 + nm[1]
    return 6 + P2I[nm[1]]


def _yblk_ref(nm):
    if nm[0] == 'y0':
        return 0
    if nm[0] == 'y1':
        return 1 + nm[1]
    return 6 + P2I[nm[1]]


# column permutations (in units of 64-blocks)
X_PERM_COLS = np.concatenate(
    [np.arange(64) + 64 * _xblk_ref(nm) for nm in X_ORDER])      # x -> x_perm
Y_INV_COLS = np.empty(DIM, dtype=np.int64)                        # y_perm -> y
for i, nm in enumerate(Y_ORDER):
    Y_INV_COLS[_yblk_ref(nm) * 64: _yblk_ref(nm) * 64 + 64] = \
        np.arange(64) + 64 * i

# aux structure: block-level 0/1 matrix over X_ORDER x AUX_ORDER
A_BLK = np.zeros((NBLK, 6), dtype=np.float32)
for i, nm in enumerate(X_ORDER):
    if nm[0] == 'x2':
        a, b = nm[1]
        A_BLK[i, AUX_ORDER.index(('T', a))] = 1
        A_BLK[i, AUX_ORDER.index(('T', b))] = 1
    elif nm[0] == 'x1':
        A_BLK[i, AUX_ORDER.index(('S1',))] = 1


def _build_Mr(weights):
    """Restructured matrix over ROW_ORDER x Y_ORDER (64-blocks), plus the
    value-independent structure set {(row_blk, col_blk)}."""
    Mr = np.zeros((len(ROW_ORDER) * 64, len(Y_ORDER) * 64), dtype=np.float32)
    structure = set()

    def add(rname, cname, blk):
        r, c = RI[rname] * 64, YI[cname] * 64
        Mr[r:r + 64, c:c + 64] += blk
        structure.add((RI[rname], YI[cname]))

    wT = {k: np.swapaxes(np.asarray(weights[k], np.float32), 1, 2)
          for k in weights}

    add(('x0',), ('y0',), wT[(0, 0)][0])
    add(('S1',), ('y0',), wT[(1, 0)][0])
    for a in range(N):
        add(('T', a), ('y0',), wT[(2, 0)][0] / 2)   # S2 == (1/2) sum_a T_a
        add(('x0',), ('y1', a), wT[(0, 1)][0])
    for p in PERM2:
        add(('x0',), ('y2', p), wT[(0, 2)][0])

    _W, widx, nbrs = KERNELS[(1, 1)]
    for a in range(N):
        row_w = {PERM1[nbrs[a, k]][0]: widx[a, k] for k in range(nbrs.shape[1])}
        offs = {row_w[p] for p in range(N) if p != a}
        assert len(offs) == 1
        woff = wT[(1, 1)][offs.pop()]
        add(('x1', a), ('y1', a), wT[(1, 1)][row_w[a]] - woff)
        add(('S1',), ('y1', a), woff)

    _W, widx, nbrs = KERNELS[(2, 1)]
    for a in range(N):
        wids = set(widx[a])
        assert len(wids) == 1
        assert {PERM2[i] for i in nbrs[a]} == {p for p in PERM2 if a in p}
        add(('T', a), ('y1', a), wT[(2, 1)][wids.pop()])

    _W, widx, nbrs = KERNELS[(1, 2)]
    for i2, (a, b) in enumerate(PERM2):
        for k in range(nbrs.shape[1]):
            c = PERM1[nbrs[i2, k]][0]
            assert c in (a, b)
            add(('x1', c), ('y2', (a, b)), wT[(1, 2)][widx[i2, k]])

    _W, widx, nbrs = KERNELS[(2, 2)]
    for i2, (a, b) in enumerate(PERM2):
        w_a = w_b = w_id = w_sw = None
        for k in range(nbrs.shape[1]):
            p1, wid = PERM2[nbrs[i2, k]], widx[i2, k]
            if p1 == (a, b):
                w_id = wid
            elif p1 == (b, a):
                w_sw = wid
            elif a in p1:
                assert b not in p1 and w_a in (None, wid)
                w_a = wid
            else:
                assert b in p1 and w_b in (None, wid)
                w_b = wid
        Wa, Wb = wT[(2, 2)][w_a], wT[(2, 2)][w_b]
        add(('x2', (a, b)), ('y2', (a, b)), wT[(2, 2)][w_id] - Wa - Wb)
        add(('x2', (b, a)), ('y2', (a, b)), wT[(2, 2)][w_sw] - Wa - Wb)
        add(('T', a), ('y2', (a, b)), Wa)
        add(('T', b), ('y2', (a, b)), Wb)
    return Mr, structure


# static schedule (value-independent): which [128,128] tiles are nonzero
def _static_schedule():
    rng_w = {k: np.random.default_rng(1).standard_normal((W, 64, 64))
             for k, (W, _, _) in KERNELS.items()}
    _, structure = _build_Mr(rng_w)
    # aux tiles: (k in 0..KT_X-1, a in 0..KT_AUX-1) with any A_BLK support
    aux_tiles = []
    for k in range(KT_X):
        for a in range(KT_AUX):
            sub = A_BLK[2 * k:2 * k + 2, 2 * a:2 * a + 2]
            if sub.any():
                aux_tiles.append((k, a))
    # main tiles: (k in 0..KT-1, o) with structure support
    main_k = [[] for _ in range(OT)]
    for (rb, cb) in structure:
        k, o = rb // 2, cb // 2
        if k not in main_k[o]:
            main_k[o].append(k)
    for o in range(OT):
        main_k[o].sort()          # x k-tiles first, aux k-tiles (13..15) last
    return aux_tiles, main_k


AUX_TILES, MAIN_K = _static_schedule()
N_TILES = len(AUX_TILES) + sum(len(v) for v in MAIN_K)


# ---------------------------------------------------------------- bass build
_CACHE = {}


def _build_bass():
    if "nc" in _CACHE:
        return _CACHE["nc"]

    from concourse import bacc, mybir, tile

    f32 = mybir.dt.float32
    f32r = mybir.dt.float32r

    nc = bacc.Bacc("TRN2", target_bir_lowering=False, debug=False,
                   num_devices=N_CORES)
    xt = nc.dram_tensor("xt", [DIM, B_SHARD], f32r, kind="ExternalInput").ap()
    mt = nc.dram_tensor("mt", [128, N_TILES * 128], f32r,
                        kind="ExternalInput").ap()
    yt = nc.dram_tensor("yt", [DIM, B_SHARD], f32, kind="ExternalOutput").ap()

    xt_r = xt.rearrange("(k p) c -> p k c", p=128)   # [128, KT_X, B_SHARD]
    yt_r = yt.rearrange("(o p) c -> p o c", p=128)   # [128, OT, B_SHARD]

    # tile index within mt for each scheduled matmul
    tidx = {}
    ti = 0
    for (k, a) in AUX_TILES:
        tidx[("aux", k, a)] = ti
        ti += 1
    for o in range(OT):
        for k in MAIN_K[o]:
            tidx[("main", k, o)] = ti
            ti += 1

    with tile.TileContext(nc) as tc:
        with (
            tc.tile_pool(name="mpool", bufs=1) as mpool,
            tc.tile_pool(name="xpool", bufs=2 * KT_X) as xpool,
            tc.tile_pool(name="apool", bufs=2 * KT_AUX) as apool,
            tc.tile_pool(name="ypool", bufs=4) as ypool,
            tc.tile_pool(name="psa", bufs=KT_AUX, space="PSUM") as psa_pool,
            tc.tile_pool(name="psm", bufs=4, space="PSUM") as psm_pool,
        ):
            # weight tiles, loaded in per-group pieces so early matmuls
            # don't wait on the whole 5.8 MB
            n_aux = len(AUX_TILES)
            m_aux = mpool.tile([128, n_aux * 128], f32r, tag="m_aux")
            nc.sync.dma_start(m_aux[:], mt[:, :n_aux * 128])
            m_o = []
            off = n_aux
            for o in range(OT):
                n_o = len(MAIN_K[o])
                t = mpool.tile([128, n_o * 128], f32r, tag=f"m_o{o}")
                nc.sync.dma_start(t[:], mt[:, off * 128:(off + n_o) * 128])
                m_o.append(t)
                off += n_o

            def lhsT(key):
                t = tidx[key]
                if key[0] == "aux":
                    return m_aux[:, t * 128:(t + 1) * 128]
                o = key[2]
                base = tidx[("main", MAIN_K[o][0], o)]
                return m_o[o][:, (t - base) * 128:(t - base + 1) * 128]

            for bc in range(BC):
                cs = slice(bc * 512, (bc + 1) * 512)
                x_sb = [xpool.tile([128, 512], f32r, tag="x", name=f"x_{bc}_{k}")
                        for k in range(KT_X)]
                for k in range(KT_X):
                    nc.sync.dma_start(x_sb[k][:], xt_r[:, k, cs])

                # aux sums via 0/1 matmuls
                aux_ps = [psa_pool.tile([128, 512], f32, tag="auxps", name=f"aps_{bc}_{a}")
                          for a in range(KT_AUX)]
                by_a = [[k for (k, a2) in AUX_TILES if a2 == a]
                        for a in range(KT_AUX)]
                for a in range(KT_AUX):
                    ks = by_a[a]
                    for i, k in enumerate(ks):
                        nc.tensor.matmul(
                            aux_ps[a][:], lhsT=lhsT(("aux", k, a)),
                            rhs=x_sb[k][:],
                            start=(i == 0), stop=(i == len(ks) - 1),
                        )
                aux_sb = []
                for a in range(KT_AUX):
                    t = apool.tile([128, 512], f32r, tag="aux", name=f"aux_{bc}_{a}")
                    nc.vector.tensor_copy(out=t[:], in_=aux_ps[a][:])
                    aux_sb.append(t)

                def rhs(k):
                    if k < KT_X:
                        return x_sb[k][:]
                    return aux_sb[k - KT_X][:]

                for o in range(OT):
                    ks = MAIN_K[o]
                    ps = psm_pool.tile([128, 512], f32, tag="ps")
                    for i, k in enumerate(ks):
                        nc.tensor.matmul(
                            ps[:], lhsT=lhsT(("main", k, o)), rhs=rhs(k),
                            start=(i == 0), stop=(i == len(ks) - 1),
                        )
                    y_sb = ypool.tile([128, 512], f32, tag="y")
                    nc.vector.tensor_copy(out=y_sb[:], in_=ps[:])
                    nc.sync.dma_start(yt_r[:, o, cs], y_sb[:])
    nc.compile()
    _CACHE["nc"] = nc
    return nc


# ---------------------------------------------------------------- entry point
def kernel(x, w_0_0, w_1_0, w_2_0, w_0_1, w_1_1, w_2_1, w_0_2, w_1_2, w_2_2,
           _trace=False):
    from concourse import bass_utils

    weights = {(0, 0): w_0_0, (1, 0): w_1_0, (2, 0): w_2_0,
               (0, 1): w_0_1, (1, 1): w_1_1, (2, 1): w_2_1,
               (0, 2): w_0_2, (1, 2): w_1_2, (2, 2): w_2_2}
    Mr, _ = _build_Mr(weights)

    # pack scheduled lhsT tiles: mt[p, t*128+q]
    AE = np.kron(A_BLK, np.eye(64, dtype=np.float32))     # [DIM, 384]
    tiles = []
    for (k, a) in AUX_TILES:
        tiles.append(AE[k * 128:(k + 1) * 128, a * 128:(a + 1) * 128])
    for o in range(OT):
        for k in MAIN_K[o]:
            tiles.append(Mr[k * 128:(k + 1) * 128, o * 128:(o + 1) * 128])
    mt_host = np.ascontiguousarray(np.concatenate(tiles, axis=1))

    x = np.asarray(x, np.float32)
    xT = np.ascontiguousarray(x[:, X_PERM_COLS].T)        # [DIM, B]

    nc = _build_bass()
    in_maps = [
        {"xt": np.ascontiguousarray(xT[:, c * B_SHARD:(c + 1) * B_SHARD]),
         "mt": mt_host}
        for c in range(N_CORES)
    ]
    res = bass_utils.run_bass_kernel_spmd(
        nc, in_maps, core_ids=list(range(N_CORES)), trace=_trace,
    )
    y = np.empty((B, DIM), dtype=np.float32)
    for c in range(N_CORES):
        y[c * B_SHARD:(c + 1) * B_SHARD, :] = \
            res.results[c]["yt"][Y_INV_COLS, :].T
    if _trace:
        kernel.last_results = res
    return y


# revision 8
# speedup vs baseline: 1.8185x; 1.1168x over previous
"""Trainium2 Bass kernel for nn_EqLinear (S5 permutation-equivariant linear layer).

The layer is y = x @ M with M [1664,1664] built by scattering the small
per-signature weight blocks.  We exploit the equivariant weight-sharing to
factor M through auxiliary sums:

    y = [x_perm, aux] @ Mr,   aux = x_perm @ A  (0/1 block sums:
        T[a] = sum of x2 perm-blocks containing a;  S1 = sum of x1 blocks)

Mr is block-sparse: only 67 of 208 [128,128] tiles are nonzero (vs 169 dense
M tiles), and A adds 21 tiles, so the tensor engine does 88 matmuls per
512-batch chunk instead of 169 — below the HBM-traffic roofline.

Sharding: batch (16384) split across 8 cores; each core runs
yT_shard = Mr.T @ [xT_shard; aux] in float32r (full-rate fp32 PE mode).
Host does only layout work (block permute / transpose / shard) plus
weight-table construction; all O(batch) FLOPs run on device.
"""

import math
import sys
from itertools import permutations

import numpy as np

for _p in ("/opt/trn_rl_repo", "/root/.axon_site/_ro/trn_rl_repo"):
    if _p not in sys.path:
        sys.path.append(_p)

# ---------------------------------------------------------------- constants
N = 5
RADIUS = 2
B = 16384
N_CORES = 8
B_SHARD = B // N_CORES          # 2048
DIM = 1664                      # (1 + 5 + 20) * 64
NBLK = DIM // 64                # 26 perm-blocks of 64 channels
KT_X = 13                       # x k-tiles (26 blocks / 2)
KT_AUX = 3                      # aux k-tiles ([T0;T1],[T2;T3],[T4;S1])
KT = KT_X + KT_AUX              # 16 total contraction tiles
OT = 13                         # output tiles (26 y-blocks / 2)
BC = B_SHARD // 512             # 4 batch chunks of 512


def _overlap(s, t):
    inv = {v: j for j, v in enumerate(t)}
    return tuple(i * 10 + inv[i] for i in s if i in inv)


def _build_kernels():
    kernels = {}
    for len2 in range(3):
        for len1 in range(3):
            if abs(len1 - len2) > RADIUS:
                continue
            wm, widx, nbrs = {}, [], []
            for perm2 in permutations(range(N), len2):
                wid, nb = [], []
                for i1, perm1 in enumerate(permutations(range(N), len1)):
                    sig = _overlap(perm1, perm2)
                    if len1 + len2 - 2 * len(sig) > RADIUS:
                        continue
                    if sig not in wm:
                        wm[sig] = len(wm)
                    wid.append(wm[sig])
                    nb.append(i1)
                widx.append(wid)
                nbrs.append(nb)
            kernels[(len1, len2)] = (len(wm), np.asarray(widx, np.int32),
                                     np.asarray(nbrs, np.int32))
    return kernels


KERNELS = _build_kernels()
PERM1 = list(permutations(range(N), 1))
PERM2 = list(permutations(range(N), 2))
P2I = {p: i for i, p in enumerate(PERM2)}

# ------------------------------------------------------------- block orders
UPAIRS = [(a, b) for a in range(N) for b in range(a + 1, N)]
X_ORDER = [('x1', 0), ('x1', 1), ('x1', 2), ('x1', 3), ('x1', 4), ('x0',)]
for (a, b) in UPAIRS:
    X_ORDER += [('x2', (a, b)), ('x2', (b, a))]
AUX_ORDER = [('T', 0), ('T', 1), ('T', 2), ('T', 3), ('T', 4), ('S1',)]
ROW_ORDER = X_ORDER + AUX_ORDER
Y_ORDER = [('y0',), ('y1', 0), ('y1', 1), ('y1', 2), ('y1', 3), ('y1', 4)]
for (a, b) in UPAIRS:
    Y_ORDER += [('y2', (a, b)), ('y2', (b, a))]

XI = {b: i for i, b in enumerate(X_ORDER)}
RI = {b: i for i, b in enumerate(ROW_ORDER)}
YI = {b: i for i, b in enumerate(Y_ORDER)}


def _xblk_ref(nm):
    if nm[0] == 'x0':
        return 0
    if nm[0] == 'x1':
        return 1 + nm[1]
    return 6 + P2I[nm[1]]


def _yblk_ref(nm):
    if nm[0] == 'y0':
        return 0
    if nm[0] == 'y1':
        return 1 + nm[1]
    return 6 + P2I[nm[1]]


# column permutations (in units of 64-blocks)
X_PERM_COLS = np.concatenate(
    [np.arange(64) + 64 * _xblk_ref(nm) for nm in X_ORDER])      # x -> x_perm
Y_INV_COLS = np.empty(DIM, dtype=np.int64)                        # y_perm -> y
for i, nm in enumerate(Y_ORDER):
    Y_INV_COLS[_yblk_ref(nm) * 64: _yblk_ref(nm) * 64 + 64] = \
        np.arange(64) + 64 * i

# aux structure: block-level 0/1 matrix over X_ORDER x AUX_ORDER
A_BLK = np.zeros((NBLK, 6), dtype=np.float32)
for i, nm in enumerate(X_ORDER):
    if nm[0] == 'x2':
        a, b = nm[1]
        A_BLK[i, AUX_ORDER.index(('T', a))] = 1
        A_BLK[i, AUX_ORDER.index(('T', b))] = 1
    elif nm[0] == 'x1':
        A_BLK[i, AUX_ORDER.index(('S1',))] = 1


def _build_Mr(weights):
    """Restructured matrix over ROW_ORDER x Y_ORDER (64-blocks), plus the
    value-independent structure set {(row_blk, col_blk)}."""
    Mr = np.zeros((len(ROW_ORDER) * 64, len(Y_ORDER) * 64), dtype=np.float32)
    structure = set()

    def add(rname, cname, blk):
        r, c = RI[rname] * 64, YI[cname] * 64
        Mr[r:r + 64, c:c + 64] += blk
        structure.add((RI[rname], YI[cname]))

    wT = {k: np.swapaxes(np.asarray(weights[k], np.float32), 1, 2)
          for k in weights}

    add(('x0',), ('y0',), wT[(0, 0)][0])
    add(('S1',), ('y0',), wT[(1, 0)][0])
    for a in range(N):
        add(('T', a), ('y0',), wT[(2, 0)][0] / 2)   # S2 == (1/2) sum_a T_a
        add(('x0',), ('y1', a), wT[(0, 1)][0])
    for p in PERM2:
        add(('x0',), ('y2', p), wT[(0, 2)][0])

    _W, widx, nbrs = KERNELS[(1, 1)]
    for a in range(N):
        row_w = {PERM1[nbrs[a, k]][0]: widx[a, k] for k in range(nbrs.shape[1])}
        offs = {row_w[p] for p in range(N) if p != a}
        assert len(offs) == 1
        woff = wT[(1, 1)][offs.pop()]
        add(('x1', a), ('y1', a), wT[(1, 1)][row_w[a]] - woff)
        add(('S1',), ('y1', a), woff)

    _W, widx, nbrs = KERNELS[(2, 1)]
    for a in range(N):
        wids = set(widx[a])
        assert len(wids) == 1
        assert {PERM2[i] for i in nbrs[a]} == {p for p in PERM2 if a in p}
        add(('T', a), ('y1', a), wT[(2, 1)][wids.pop()])

    _W, widx, nbrs = KERNELS[(1, 2)]
    for i2, (a, b) in enumerate(PERM2):
        for k in range(nbrs.shape[1]):
            c = PERM1[nbrs[i2, k]][0]
            assert c in (a, b)
            add(('x1', c), ('y2', (a, b)), wT[(1, 2)][widx[i2, k]])

    _W, widx, nbrs = KERNELS[(2, 2)]
    for i2, (a, b) in enumerate(PERM2):
        w_a = w_b = w_id = w_sw = None
        for k in range(nbrs.shape[1]):
            p1, wid = PERM2[nbrs[i2, k]], widx[i2, k]
            if p1 == (a, b):
                w_id = wid
            elif p1 == (b, a):
                w_sw = wid
            elif a in p1:
                assert b not in p1 and w_a in (None, wid)
                w_a = wid
            else:
                assert b in p1 and w_b in (None, wid)
                w_b = wid
        Wa, Wb = wT[(2, 2)][w_a], wT[(2, 2)][w_b]
        add(('x2', (a, b)), ('y2', (a, b)), wT[(2, 2)][w_id] - Wa - Wb)
        add(('x2', (b, a)), ('y2', (a, b)), wT[(2, 2)][w_sw] - Wa - Wb)
        add(('T', a), ('y2', (a, b)), Wa)
        add(('T', b), ('y2', (a, b)), Wb)
    return Mr, structure


# static schedule (value-independent): which [128,128] tiles are nonzero
def _static_schedule():
    rng_w = {k: np.random.default_rng(1).standard_normal((W, 64, 64))
             for k, (W, _, _) in KERNELS.items()}
    _, structure = _build_Mr(rng_w)
    # aux tiles: (k in 0..KT_X-1, a in 0..KT_AUX-1) with any A_BLK support
    aux_tiles = []
    for k in range(KT_X):
        for a in range(KT_AUX):
            sub = A_BLK[2 * k:2 * k + 2, 2 * a:2 * a + 2]
            if sub.any():
                aux_tiles.append((k, a))
    # main tiles: (k in 0..KT-1, o) with structure support
    main_k = [[] for _ in range(OT)]
    for (rb, cb) in structure:
        k, o = rb // 2, cb // 2
        if k not in main_k[o]:
            main_k[o].append(k)
    for o in range(OT):
        main_k[o].sort()          # x k-tiles first, aux k-tiles (13..15) last
    return aux_tiles, main_k


AUX_TILES, MAIN_K = _static_schedule()
# dedup structurally-identical aux 0/1 tiles (same [2,2] block pattern)
_AUX_PAT = {}
AUX_UNIQ = []          # list of (k, a) representatives, defines mt aux section
AUX_TIDX = {}          # (k, a) -> index into AUX_UNIQ
for (k, a) in AUX_TILES:
    pat = tuple(A_BLK[2 * k:2 * k + 2, 2 * a:2 * a + 2].ravel().astype(int))
    if pat not in _AUX_PAT:
        _AUX_PAT[pat] = len(AUX_UNIQ)
        AUX_UNIQ.append((k, a))
    AUX_TIDX[(k, a)] = _AUX_PAT[pat]
N_TILES = len(AUX_UNIQ) + sum(len(v) for v in MAIN_K)


# ---------------------------------------------------------------- bass build
_CACHE = {}


def _build_bass():
    if "nc" in _CACHE:
        return _CACHE["nc"]

    from concourse import bacc, mybir, tile

    f32 = mybir.dt.float32
    f32r = mybir.dt.float32r

    nc = bacc.Bacc("TRN2", target_bir_lowering=False, debug=False,
                   num_devices=N_CORES)
    xt = nc.dram_tensor("xt", [DIM, B_SHARD], f32r, kind="ExternalInput").ap()
    mt = nc.dram_tensor("mt", [128, N_TILES * 128], f32r,
                        kind="ExternalInput").ap()
    yt = nc.dram_tensor("yt", [DIM, B_SHARD], f32, kind="ExternalOutput").ap()

    xt_r = xt.rearrange("(k p) c -> p k c", p=128)   # [128, KT_X, B_SHARD]
    yt_r = yt.rearrange("(o p) c -> p o c", p=128)   # [128, OT, B_SHARD]

    # tile index within mt for each scheduled matmul
    tidx = {}
    for (k, a) in AUX_TILES:
        tidx[("aux", k, a)] = AUX_TIDX[(k, a)]
    ti = len(AUX_UNIQ)
    for o in range(OT):
        for k in MAIN_K[o]:
            tidx[("main", k, o)] = ti
            ti += 1

    with tile.TileContext(nc) as tc:
        with (
            tc.tile_pool(name="mpool", bufs=1) as mpool,
            tc.tile_pool(name="xpool", bufs=2 * KT_X) as xpool,
            tc.tile_pool(name="apool", bufs=2 * KT_AUX) as apool,
            tc.tile_pool(name="ypool", bufs=4) as ypool,
            tc.tile_pool(name="psa", bufs=KT_AUX, space="PSUM") as psa_pool,
            tc.tile_pool(name="psm", bufs=5, space="PSUM") as psm_pool,
        ):
            # weight tiles, loaded in per-group pieces so early matmuls
            # don't wait on the whole 5.8 MB
            n_aux = len(AUX_UNIQ)
            m_aux = mpool.tile([128, n_aux * 128], f32r, tag="m_aux")
            nc.scalar.dma_start(m_aux[:], mt[:, :n_aux * 128])
            m_o = []
            off = n_aux
            for o in range(OT):
                n_o = len(MAIN_K[o])
                t = mpool.tile([128, n_o * 128], f32r, tag=f"m_o{o}")
                nc.scalar.dma_start(t[:], mt[:, off * 128:(off + n_o) * 128])
                m_o.append(t)
                off += n_o

            def lhsT(key):
                t = tidx[key]
                if key[0] == "aux":
                    return m_aux[:, t * 128:(t + 1) * 128]
                o = key[2]
                base = tidx[("main", MAIN_K[o][0], o)]
                return m_o[o][:, (t - base) * 128:(t - base + 1) * 128]

            for bc in range(BC):
                cs = slice(bc * 512, (bc + 1) * 512)
                x_sb = [xpool.tile([128, 512], f32r, tag="x", name=f"x_{bc}_{k}")
                        for k in range(KT_X)]
                for k in range(KT_X):
                    nc.sync.dma_start(x_sb[k][:], xt_r[:, k, cs])

                # aux sums via 0/1 matmuls
                aux_ps = [psa_pool.tile([128, 512], f32, tag="auxps", name=f"aps_{bc}_{a}")
                          for a in range(KT_AUX)]
                by_a = [[k for (k, a2) in AUX_TILES if a2 == a]
                        for a in range(KT_AUX)]
                for a in range(KT_AUX):
                    ks = by_a[a]
                    for i, k in enumerate(ks):
                        nc.tensor.matmul(
                            aux_ps[a][:], lhsT=lhsT(("aux", k, a)),
                            rhs=x_sb[k][:],
                            start=(i == 0), stop=(i == len(ks) - 1),
                        )
                aux_sb = []
                for a in range(KT_AUX):
                    t = apool.tile([128, 512], f32r, tag="aux", name=f"aux_{bc}_{a}")
                    nc.vector.tensor_copy(out=t[:], in_=aux_ps[a][:])
                    aux_sb.append(t)

                def rhs(k):
                    if k < KT_X:
                        return x_sb[k][:]
                    return aux_sb[k - KT_X][:]

                for o in range(OT):
                    ks = MAIN_K[o]
                    ps = psm_pool.tile([128, 512], f32, tag="ps")
                    for i, k in enumerate(ks):
                        nc.tensor.matmul(
                            ps[:], lhsT=lhsT(("main", k, o)), rhs=rhs(k),
                            start=(i == 0), stop=(i == len(ks) - 1),
                        )
                    y_sb = ypool.tile([128, 512], f32, tag="y")
                    nc.scalar.copy(out=y_sb[:], in_=ps[:])
                    nc.scalar.dma_start(yt_r[:, o, cs], y_sb[:])
    nc.compile()
    _CACHE["nc"] = nc
    return nc


# ---------------------------------------------------------------- entry point
def kernel(x, w_0_0, w_1_0, w_2_0, w_0_1, w_1_1, w_2_1, w_0_2, w_1_2, w_2_2,
           _trace=False):
    from concourse import bass_utils

    weights = {(0, 0): w_0_0, (1, 0): w_1_0, (2, 0): w_2_0,
               (0, 1): w_0_1, (1, 1): w_1_1, (2, 1): w_2_1,
               (0, 2): w_0_2, (1, 2): w_1_2, (2, 2): w_2_2}
    Mr, _ = _build_Mr(weights)

    # pack scheduled lhsT tiles: mt[p, t*128+q]
    AE = np.kron(A_BLK, np.eye(64, dtype=np.float32))     # [DIM, 384]
    tiles = []
    for (k, a) in AUX_UNIQ:
        tiles.append(AE[k * 128:(k + 1) * 128, a * 128:(a + 1) * 128])
    for o in range(OT):
        for k in MAIN_K[o]:
            tiles.append(Mr[k * 128:(k + 1) * 128, o * 128:(o + 1) * 128])
    mt_host = np.ascontiguousarray(np.concatenate(tiles, axis=1))

    x = np.asarray(x, np.float32)
    xT = np.ascontiguousarray(x[:, X_PERM_COLS].T)        # [DIM, B]

    nc = _build_bass()
    in_maps = [
        {"xt": np.ascontiguousarray(xT[:, c * B_SHARD:(c + 1) * B_SHARD]),
         "mt": mt_host}
        for c in range(N_CORES)
    ]
    res = bass_utils.run_bass_kernel_spmd(
        nc, in_maps, core_ids=list(range(N_CORES)), trace=_trace,
    )
    y = np.empty((B, DIM), dtype=np.float32)
    for c in range(N_CORES):
        y[c * B_SHARD:(c + 1) * B_SHARD, :] = \
            res.results[c]["yt"][Y_INV_COLS, :].T
    if _trace:
        kernel.last_results = res
    return y
